# revision 29
# baseline (speedup 1.0000x reference)
"""Discounted cumsum (y[b,h,t,d] = x[b,h,t,d] + gamma[h] * y[b,h,t-1,d]) on 8 trn2 cores.

Blocked parallel scan, pure data parallelism over the B*H=64 (b,h) pairs (8 per core).
SBUF layout per pair: [128 part = t-within-block, 32 blocks x 128 d]. Everything runs
at 2 bytes/element (input cast to bf16 on host, output written back as bf16 in the
scan layout and un-permuted/upcast on host), so the kernel sits on the ~330 GB/s
per-core DMA roofline with fully contiguous 1 MiB transfers both ways. fp32 PSUM
accumulation keeps the end-to-end error ~2^-9 relative, far inside the 2e-2 gate.

The cross-block carry needs NO sequential chain: state decays by gamma^128 ~ 6e-8
per block, so only the immediately preceding block contributes (guarded host-side;
a second shift term is added if any gamma^128 exceeds 1e-4). Block k's output is
two accumulating PE matmuls over the same PSUM group:

    y[:, k] = A^T x[:, k] + B^T x[:, k-1]
    A[s,t] = gamma^(t-s) (t>=s),   B[s,t] = gamma^(t+128-s)

i.e. the carry rides the scan matmul as a second pass with the moving operand
shifted one block left. No block sums, no carry matmul, no row patching, no
SBUF round-trips - each pair is load -> 16 matmuls -> 8 PSUM->SBUF copies
(alternating vector/scalar engines) -> store, and pairs are independent, so the
machine stays packed: input halves stream on the sync+scalar queues, each output
column-half ships as soon as its copies land (queues alternate per pair).
"""

import numpy as np

B, H, S, D = 4, 16, 4096, 128
T = 128          # block length (matmul contraction dim)
KB = S // T      # 32 blocks per sequence
NG = 4           # blocks per scan-matmul group (4*128 = 512 moving columns)
G = KB // NG     # 8 scan groups per pair
NCORES = 8
PAIRS = (B * H) // NCORES  # 8 pair-slots per core

_nc_cache = {}


def _build_program(nshift):
    """nshift: how many previous blocks feed the carry (1 unless gamma ~ 1)."""
    if nshift in _nc_cache:
        return _nc_cache[nshift]

    import concourse.bass as bass
    import concourse.mybir as mybir
    from concourse.tile import TileContext

    bf16 = mybir.dt.bfloat16
    f32 = mybir.dt.float32

    nc = bass.Bass(trn_type="TRN2")

    NM = nshift + 1  # stationaries per pair (A + shift mats), packed per pair
    x_d = nc.declare_dram_parameter("x_all", [PAIRS, T, KB * D], bf16, isOutput=False)
    AB_d = nc.declare_dram_parameter(
        "AB_all", [T, PAIRS * NM * T], bf16, isOutput=False
    )
    y_d = nc.declare_dram_parameter("y", [PAIRS, T, KB * D], bf16, isOutput=True)

    with TileContext(nc) as tc:
        with (
            tc.tile_pool(name="const", bufs=1) as cpool,
            tc.tile_pool(name="xin", bufs=PAIRS) as xpool,
            tc.tile_pool(name="yout", bufs=3) as ypool,
            tc.tile_pool(name="grp_ps", bufs=6, space="PSUM") as gp_pool,
        ):
            ABc = cpool.tile([T, PAIRS * NM * T], bf16, tag="ABc")

            def absorb(ap_src):
                # standalone bf16 ldweights: makes PE wait on that tile's DMA
                # lane here; the real matmuls self-load their own stationary.
                nc.tensor.ldweights(ap_src.bitcast(bf16))

            # ---- constants in one DMA (512B descriptors, ~1.5us), then all
            # input loads up front, full 1 MiB DMAs alternating the two HWDGE
            # queues (big transfers sustain the best rate)
            nc.sync.dma_start(out=ABc[:], in_=AB_d[:])
            absorb(ABc[0:1, 0:1])
            Xs = []
            for p in range(PAIRS):
                X = xpool.tile([T, KB * D], bf16, tag="X")
                eng = nc.sync if p % 2 == 0 else nc.scalar
                eng.dma_start(out=X[:], in_=x_d[p])
                absorb(X[0:1, 0:1])
                Xs.append(X)

            for p in range(PAIRS):
                X = Xs[p]
                Ys = ypool.tile([T, KB * D], bf16, tag="Ys")
                for g in range(G):
                    grp = gp_pool.tile([T, NG * D], f32, tag="grp")
                    lo = g * NG * D
                    hi = (g + 1) * NG * D
                    base = p * NM * T
                    nc.tensor.matmul(
                        grp[:], lhsT=ABc[:, base : base + T],
                        rhs=X[:, lo:hi], start=True, stop=True,
                        skip_group_check=True,
                    )
                    for j in range(1, nshift + 1):
                        # carry from the j-th previous block: moving operand
                        # shifted j*D columns left; block indices < j get none
                        off = max(0, j * D - lo)
                        nc.tensor.matmul(
                            grp[:, off:],
                            lhsT=ABc[:, base + j * T : base + (j + 1) * T],
                            rhs=X[:, lo + off - j * D : hi - j * D],
                            start=False, stop=False,
                            skip_group_check=True,
                        )
                    if g % 2 == 0:
                        nc.vector.tensor_copy(out=Ys[:, lo:hi], in_=grp[:])
                    else:
                        nc.scalar.copy(out=Ys[:, lo:hi], in_=grp[:])
                if p == PAIRS - 1:
                    # last pair: halve the store across both queues (tail)
                    hh = KB * D // 2
                    nc.scalar.dma_start(out=y_d[p][:, 0:hh], in_=Ys[:, 0:hh])
                    nc.sync.dma_start(out=y_d[p][:, hh:], in_=Ys[:, hh:])
                else:
                    # full 1 MiB store on the opposite queue from the load
                    eng = nc.scalar if p % 2 == 0 else nc.sync
                    eng.dma_start(out=y_d[p], in_=Ys[:])

    # Split excess per-instruction sync waits onto InstEventSemaphore carriers.
    import bass_rust

    bass_rust.generate_event_semaphores(nc)

    _nc_cache[nshift] = nc
    return nc


def _host_constants(g, nshift):
    """A (in-block scan) and the shifted-carry matrices from f64 gamma powers."""
    pw = np.power(g, np.arange((nshift + 2) * T, dtype=np.float64))
    t_idx = np.arange(T)
    t_minus_s = t_idx[None, :] - t_idx[:, None]
    mats = [np.where(t_minus_s >= 0, pw[np.clip(t_minus_s, 0, None)], 0.0)]
    for j in range(1, nshift + 1):
        mats.append(pw[t_minus_s + j * T])
    return mats


def _make_in_maps(tensor, gamma):
    import ml_dtypes

    bf16 = ml_dtypes.bfloat16
    x = np.ascontiguousarray(np.asarray(tensor, dtype=np.float32)).reshape(
        B * H, S, D
    )
    gam = np.asarray(gamma, dtype=np.float64).reshape(H)
    # one shift term per gamma^128 decade above negligibility (1 for seed-0 data)
    nshift = 1
    while np.max(gam) ** (nshift * T) > 1e-4:
        nshift += 1

    in_maps = []
    for c in range(NCORES):
        xa = np.empty((PAIRS, T, KB * D), bf16)
        NM = nshift + 1
        AB_all = np.empty((T, PAIRS * NM * T), bf16)
        for p in range(PAIRS):
            pid = c * PAIRS + p
            mats = _host_constants(gam[pid % H], nshift)
            # x in scan layout [s, (k, d)]
            xa[p] = (
                x[pid]
                .reshape(KB, T, D)
                .transpose(1, 0, 2)
                .reshape(T, KB * D)
                .astype(bf16)
            )
            for j, m in enumerate(mats):
                AB_all[:, (p * NM + j) * T : (p * NM + j + 1) * T] = m.astype(
                    bf16
                )
        in_maps.append({"x_all": xa, "AB_all": AB_all})
    return in_maps, nshift


def kernel(tensor, gamma):
    from concourse.bass_utils import run_bass_kernel_spmd

    in_maps, nshift = _make_in_maps(tensor, gamma)
    nc = _build_program(nshift)
    res = run_bass_kernel_spmd(nc, in_maps, list(range(NCORES))).results
    y = np.empty((B * H, S, D), np.float32)
    for c in range(NCORES):
        yc = np.asarray(res[c]["y"]).astype(np.float32)
        y[c * PAIRS : (c + 1) * PAIRS] = (
            yc.reshape(PAIRS, T, KB, D).transpose(0, 2, 1, 3).reshape(PAIRS, S, D)
        )
    return y.reshape(B, H, S, D)


# revision 30
# speedup vs baseline: 1.0865x; 1.0865x over previous
"""Discounted cumsum (y[b,h,t,d] = x[b,h,t,d] + gamma[h] * y[b,h,t-1,d]) on 8 trn2 cores.

Blocked parallel scan, pure data parallelism over the B*H=64 (b,h) pairs (8 per core).
SBUF layout per pair: [128 part = t-within-block, 32 blocks x 128 d]. Everything runs
at 2 bytes/element (input cast to bf16 on host, output written back as bf16 in the
scan layout and un-permuted/upcast on host), so the kernel sits on the ~330 GB/s
per-core DMA roofline with fully contiguous 1 MiB transfers both ways. fp32 PSUM
accumulation keeps the end-to-end error ~2^-9 relative, far inside the 2e-2 gate.

The cross-block carry needs NO sequential chain: state decays by gamma^128 ~ 6e-8
per block, so only the immediately preceding block contributes (guarded host-side;
a second shift term is added if any gamma^128 exceeds 1e-4). Block k's output is
two accumulating PE matmuls over the same PSUM group:

    y[:, k] = A^T x[:, k] + B^T x[:, k-1]
    A[s,t] = gamma^(t-s) (t>=s),   B[s,t] = gamma^(t+128-s)

i.e. the carry rides the scan matmul as a second pass with the moving operand
shifted one block left. No block sums, no carry matmul, no row patching, no
SBUF round-trips - each pair is load -> 16 matmuls -> 8 PSUM->SBUF copies
(alternating vector/scalar engines) -> store, and pairs are independent, so the
machine stays packed: input halves stream on the sync+scalar queues, each output
column-half ships as soon as its copies land (queues alternate per pair).
"""

import numpy as np

B, H, S, D = 4, 16, 4096, 128
T = 128          # block length (matmul contraction dim)
KB = S // T      # 32 blocks per sequence
NG = 4           # blocks per scan-matmul group (4*128 = 512 moving columns)
G = KB // NG     # 8 scan groups per pair
NCORES = 8
PAIRS = (B * H) // NCORES  # 8 pair-slots per core

_nc_cache = {}


def _build_program(nshift):
    """nshift: how many previous blocks feed the carry (1 unless gamma ~ 1)."""
    if nshift in _nc_cache:
        return _nc_cache[nshift]

    import concourse.bass as bass
    import concourse.mybir as mybir
    from concourse.tile import TileContext

    bf16 = mybir.dt.bfloat16
    f32 = mybir.dt.float32

    nc = bass.Bass(trn_type="TRN2")

    NM = nshift + 1  # stationaries per pair (A + shift mats), packed per pair
    x_d = nc.declare_dram_parameter("x_all", [PAIRS, T, KB * D], bf16, isOutput=False)
    AB_d = nc.declare_dram_parameter(
        "AB_all", [T, PAIRS * NM * T], bf16, isOutput=False
    )
    y_d = nc.declare_dram_parameter("y", [PAIRS, T, KB * D], bf16, isOutput=True)

    with TileContext(nc) as tc:
        with (
            tc.tile_pool(name="const", bufs=1) as cpool,
            tc.tile_pool(name="xin", bufs=PAIRS) as xpool,
            tc.tile_pool(name="yout", bufs=3) as ypool,
            tc.tile_pool(name="grp_ps", bufs=6, space="PSUM") as gp_pool,
        ):
            ABc = cpool.tile([T, PAIRS * NM * T], bf16, tag="ABc")

            def absorb(ap_src):
                # standalone bf16 ldweights: makes PE wait on that tile's DMA
                # lane here; the real matmuls self-load their own stationary.
                nc.tensor.ldweights(ap_src.bitcast(bf16))

            # ---- constants in one DMA (512B descriptors, ~1.5us), then all
            # input loads up front, full 1 MiB DMAs alternating the two HWDGE
            # queues (big transfers sustain the best rate)
            nc.sync.dma_start(out=ABc[:], in_=AB_d[:])
            absorb(ABc[0:1, 0:1])
            Xs = []
            for p in range(PAIRS):
                X = xpool.tile([T, KB * D], bf16, tag="X")
                eng = nc.sync if p % 2 == 0 else nc.scalar
                eng.dma_start(out=X[:], in_=x_d[p])
                absorb(X[0:1, 0:1])
                Xs.append(X)

            for p in range(PAIRS):
                X = Xs[p]
                Ys = ypool.tile([T, KB * D], bf16, tag="Ys")
                for g in range(G):
                    grp = gp_pool.tile([T, NG * D], f32, tag="grp")
                    lo = g * NG * D
                    hi = (g + 1) * NG * D
                    base = p * NM * T
                    nc.tensor.matmul(
                        grp[:], lhsT=ABc[:, base : base + T],
                        rhs=X[:, lo:hi], start=True, stop=True,
                        skip_group_check=True,
                    )
                    for j in range(1, nshift + 1):
                        # carry from the j-th previous block: moving operand
                        # shifted j*D columns left; block indices < j get none
                        off = max(0, j * D - lo)
                        nc.tensor.matmul(
                            grp[:, off:],
                            lhsT=ABc[:, base + j * T : base + (j + 1) * T],
                            rhs=X[:, lo + off - j * D : hi - j * D],
                            start=False, stop=False,
                            skip_group_check=True,
                        )
                    if g % 2 == 0:
                        nc.vector.tensor_copy(out=Ys[:, lo:hi], in_=grp[:])
                    else:
                        nc.scalar.copy(out=Ys[:, lo:hi], in_=grp[:])
                if p >= PAIRS - 2:
                    # last pairs: halve the store across both queues (tail)
                    hh = KB * D // 2
                    nc.scalar.dma_start(out=y_d[p][:, 0:hh], in_=Ys[:, 0:hh])
                    nc.sync.dma_start(out=y_d[p][:, hh:], in_=Ys[:, hh:])
                else:
                    # full 1 MiB store on the opposite queue from the load
                    eng = nc.scalar if p % 2 == 0 else nc.sync
                    eng.dma_start(out=y_d[p], in_=Ys[:])

    # Split excess per-instruction sync waits onto InstEventSemaphore carriers.
    import bass_rust

    bass_rust.generate_event_semaphores(nc)

    _nc_cache[nshift] = nc
    return nc


def _host_constants(g, nshift):
    """A (in-block scan) and the shifted-carry matrices from f64 gamma powers."""
    pw = np.power(g, np.arange((nshift + 2) * T, dtype=np.float64))
    t_idx = np.arange(T)
    t_minus_s = t_idx[None, :] - t_idx[:, None]
    mats = [np.where(t_minus_s >= 0, pw[np.clip(t_minus_s, 0, None)], 0.0)]
    for j in range(1, nshift + 1):
        mats.append(pw[t_minus_s + j * T])
    return mats


def _make_in_maps(tensor, gamma):
    import ml_dtypes

    bf16 = ml_dtypes.bfloat16
    x = np.ascontiguousarray(np.asarray(tensor, dtype=np.float32)).reshape(
        B * H, S, D
    )
    gam = np.asarray(gamma, dtype=np.float64).reshape(H)
    # one shift term per gamma^128 decade above negligibility (1 for seed-0 data)
    nshift = 1
    while np.max(gam) ** (nshift * T) > 1e-4:
        nshift += 1

    in_maps = []
    for c in range(NCORES):
        xa = np.empty((PAIRS, T, KB * D), bf16)
        NM = nshift + 1
        AB_all = np.empty((T, PAIRS * NM * T), bf16)
        for p in range(PAIRS):
            pid = c * PAIRS + p
            mats = _host_constants(gam[pid % H], nshift)
            # x in scan layout [s, (k, d)]
            xa[p] = (
                x[pid]
                .reshape(KB, T, D)
                .transpose(1, 0, 2)
                .reshape(T, KB * D)
                .astype(bf16)
            )
            for j, m in enumerate(mats):
                AB_all[:, (p * NM + j) * T : (p * NM + j + 1) * T] = m.astype(
                    bf16
                )
        in_maps.append({"x_all": xa, "AB_all": AB_all})
    return in_maps, nshift


def kernel(tensor, gamma):
    from concourse.bass_utils import run_bass_kernel_spmd

    in_maps, nshift = _make_in_maps(tensor, gamma)
    nc = _build_program(nshift)
    res = run_bass_kernel_spmd(nc, in_maps, list(range(NCORES))).results
    y = np.empty((B * H, S, D), np.float32)
    for c in range(NCORES):
        yc = np.asarray(res[c]["y"]).astype(np.float32)
        y[c * PAIRS : (c + 1) * PAIRS] = (
            yc.reshape(PAIRS, T, KB, D).transpose(0, 2, 1, 3).reshape(PAIRS, S, D)
        )
    return y.reshape(B, H, S, D)


# revision 31
# speedup vs baseline: 1.1346x; 1.0443x over previous
"""Discounted cumsum (y[b,h,t,d] = x[b,h,t,d] + gamma[h] * y[b,h,t-1,d]) on 8 trn2 cores.

Blocked parallel scan, pure data parallelism over the B*H=64 (b,h) pairs (8 per core).
SBUF layout per pair: [128 part = t-within-block, 32 blocks x 128 d]. Everything runs
at 2 bytes/element (input cast to bf16 on host, output written back as bf16 in the
scan layout and un-permuted/upcast on host), so the kernel sits on the ~330 GB/s
per-core DMA roofline with fully contiguous 1 MiB transfers both ways. fp32 PSUM
accumulation keeps the end-to-end error ~2^-9 relative, far inside the 2e-2 gate.

The cross-block carry needs NO sequential chain: state decays by gamma^128 ~ 6e-8
per block, so only the immediately preceding block contributes (guarded host-side;
a second shift term is added if any gamma^128 exceeds 1e-4). Block k's output is
two accumulating PE matmuls over the same PSUM group:

    y[:, k] = A^T x[:, k] + B^T x[:, k-1]
    A[s,t] = gamma^(t-s) (t>=s),   B[s,t] = gamma^(t+128-s)

i.e. the carry rides the scan matmul as a second pass with the moving operand
shifted one block left. No block sums, no carry matmul, no row patching, no
SBUF round-trips - each pair is load -> 16 matmuls -> 8 PSUM->SBUF copies
(alternating vector/scalar engines) -> store, and pairs are independent, so the
machine stays packed: input halves stream on the sync+scalar queues, each output
column-half ships as soon as its copies land (queues alternate per pair).
"""

import numpy as np

B, H, S, D = 4, 16, 4096, 128
T = 128          # block length (matmul contraction dim)
KB = S // T      # 32 blocks per sequence
NG = 4           # blocks per scan-matmul group (4*128 = 512 moving columns)
G = KB // NG     # 8 scan groups per pair
NCORES = 8
PAIRS = (B * H) // NCORES  # 8 pair-slots per core

_nc_cache = {}


def _build_program(nshift):
    """nshift: how many previous blocks feed the carry (1 unless gamma ~ 1)."""
    if nshift in _nc_cache:
        return _nc_cache[nshift]

    import concourse.bass as bass
    import concourse.mybir as mybir
    from concourse.tile import TileContext

    bf16 = mybir.dt.bfloat16
    f32 = mybir.dt.float32

    nc = bass.Bass(trn_type="TRN2")

    NM = nshift + 1  # stationaries per pair (A + shift mats), packed per pair
    x_d = nc.declare_dram_parameter("x_all", [PAIRS, T, KB * D], bf16, isOutput=False)
    AB_d = nc.declare_dram_parameter(
        "AB_all", [T, PAIRS * NM * T], bf16, isOutput=False
    )
    y_d = nc.declare_dram_parameter("y", [PAIRS, T, KB * D], bf16, isOutput=True)

    with TileContext(nc) as tc:
        with (
            tc.tile_pool(name="const", bufs=1) as cpool,
            tc.tile_pool(name="xin", bufs=PAIRS) as xpool,
            tc.tile_pool(name="yout", bufs=3) as ypool,
            tc.tile_pool(name="grp_ps", bufs=6, space="PSUM") as gp_pool,
        ):
            ABc = cpool.tile([T, PAIRS * NM * T], bf16, tag="ABc")

            def absorb(ap_src):
                # standalone bf16 ldweights: makes PE wait on that tile's DMA
                # lane here; the real matmuls self-load their own stationary.
                nc.tensor.ldweights(ap_src.bitcast(bf16))

            # ---- constants in one DMA (512B descriptors, ~1.5us), then all
            # input loads up front, full 1 MiB DMAs alternating the two HWDGE
            # queues (big transfers sustain the best rate)
            nc.sync.dma_start(out=ABc[:], in_=AB_d[:])
            absorb(ABc[0:1, 0:1])
            Xs = []
            for p in range(PAIRS):
                X = xpool.tile([T, KB * D], bf16, tag="X")
                eng = nc.sync if p % 2 == 0 else nc.scalar
                eng.dma_start(out=X[:], in_=x_d[p])
                absorb(X[0:1, 0:1])
                Xs.append(X)

            for p in range(PAIRS):
                X = Xs[p]
                Ys = ypool.tile([T, KB * D], bf16, tag="Ys")
                for g in range(G):
                    grp = gp_pool.tile([T, NG * D], f32, tag="grp")
                    lo = g * NG * D
                    hi = (g + 1) * NG * D
                    base = p * NM * T
                    nc.tensor.matmul(
                        grp[:], lhsT=ABc[:, base : base + T],
                        rhs=X[:, lo:hi], start=True, stop=True,
                        skip_group_check=True,
                    )
                    for j in range(1, nshift + 1):
                        # carry from the j-th previous block: moving operand
                        # shifted j*D columns left; block indices < j get none
                        off = max(0, j * D - lo)
                        nc.tensor.matmul(
                            grp[:, off:],
                            lhsT=ABc[:, base + j * T : base + (j + 1) * T],
                            rhs=X[:, lo + off - j * D : hi - j * D],
                            start=False, stop=False,
                            skip_group_check=True,
                        )
                    if g % 2 == 0:
                        nc.vector.tensor_copy(out=Ys[:, lo:hi], in_=grp[:])
                    else:
                        nc.scalar.copy(out=Ys[:, lo:hi], in_=grp[:])
                # all stores issue from the sync engine: the scalar engine's
                # in-order stream must not stall on store-issue waits, or its
                # copies lag and PSUM recycling blocks the PE
                nc.sync.dma_start(out=y_d[p], in_=Ys[:])

    # Split excess per-instruction sync waits onto InstEventSemaphore carriers.
    import bass_rust

    bass_rust.generate_event_semaphores(nc)

    _nc_cache[nshift] = nc
    return nc


def _host_constants(g, nshift):
    """A (in-block scan) and the shifted-carry matrices from f64 gamma powers."""
    pw = np.power(g, np.arange((nshift + 2) * T, dtype=np.float64))
    t_idx = np.arange(T)
    t_minus_s = t_idx[None, :] - t_idx[:, None]
    mats = [np.where(t_minus_s >= 0, pw[np.clip(t_minus_s, 0, None)], 0.0)]
    for j in range(1, nshift + 1):
        mats.append(pw[t_minus_s + j * T])
    return mats


def _make_in_maps(tensor, gamma):
    import ml_dtypes

    bf16 = ml_dtypes.bfloat16
    x = np.ascontiguousarray(np.asarray(tensor, dtype=np.float32)).reshape(
        B * H, S, D
    )
    gam = np.asarray(gamma, dtype=np.float64).reshape(H)
    # one shift term per gamma^128 decade above negligibility (1 for seed-0 data)
    nshift = 1
    while np.max(gam) ** (nshift * T) > 1e-4:
        nshift += 1

    in_maps = []
    for c in range(NCORES):
        xa = np.empty((PAIRS, T, KB * D), bf16)
        NM = nshift + 1
        AB_all = np.empty((T, PAIRS * NM * T), bf16)
        for p in range(PAIRS):
            pid = c * PAIRS + p
            mats = _host_constants(gam[pid % H], nshift)
            # x in scan layout [s, (k, d)]
            xa[p] = (
                x[pid]
                .reshape(KB, T, D)
                .transpose(1, 0, 2)
                .reshape(T, KB * D)
                .astype(bf16)
            )
            for j, m in enumerate(mats):
                AB_all[:, (p * NM + j) * T : (p * NM + j + 1) * T] = m.astype(
                    bf16
                )
        in_maps.append({"x_all": xa, "AB_all": AB_all})
    return in_maps, nshift


def kernel(tensor, gamma):
    from concourse.bass_utils import run_bass_kernel_spmd

    in_maps, nshift = _make_in_maps(tensor, gamma)
    nc = _build_program(nshift)
    res = run_bass_kernel_spmd(nc, in_maps, list(range(NCORES))).results
    y = np.empty((B * H, S, D), np.float32)
    for c in range(NCORES):
        yc = np.asarray(res[c]["y"]).astype(np.float32)
        y[c * PAIRS : (c + 1) * PAIRS] = (
            yc.reshape(PAIRS, T, KB, D).transpose(0, 2, 1, 3).reshape(PAIRS, S, D)
        )
    return y.reshape(B, H, S, D)
